# revision 1
# baseline (speedup 1.0000x reference)
"""Causal self-attention (B=2, S=2048, E=1024, H=16) on 8 trn2 cores.

Sharding: batch x head -- core c handles batch c//4 and the 4 heads
starting at (c%4)*4. Each core runs QKV projection for its heads,
causal attention, and its slice of the output projection (row-split
c_proj); the host sums the 4 partial projections per batch.

v2: single merged pipeline. The QKV projection (PE-heavy, ACT-idle)
and the attention inner loop (ACT-heavy on the exp) are interleaved in
one emission stream so the scalar engine's exp work hides under
projection/attention matmuls instead of serializing behind them.
Normalization drops the Ln/Exp reciprocal chain: the raw rowsum is
broadcast with a ones-matmul and DVE divides directly. Output partials
are written fp16 and spread across DMA queues; inputs stream on four
queues in consumption order as multi-chunk transfers.

Layout trick: scores are computed transposed (S^T[k, q]) so every
matmul streams 512 moving columns, and the attention output comes
out as y^T[d, q] -- exactly the stationary operand the output
projection needs. Row-sums ride along as a ones-column appended to V.
All fp32 matmul inputs are float32r (~1e-4 rel err, full PE rate at
moving >= 256).
"""

import os
import sys

import numpy as np

_DIR = os.path.dirname(os.path.abspath(__file__))
for _p in (_DIR,):
    if _p not in sys.path:
        sys.path.insert(0, _p)

import concourse.bass as bass
import concourse.mybir as mybir
from concourse import tile
from concourse.vector_clock import ScopedClock, VectorClock

F32 = mybir.dt.float32
F32R = mybir.dt.float32r
F16 = mybir.dt.float16
U16 = mybir.dt.uint16

B, S, E, H, D = 2, 2048, 1024, 16, 64
HPC = 4          # heads per core
N_CORES = 8
QT = 512         # q tile (moving dim)
KC = 128         # k chunk (contraction tile)
EC = E // 128    # 8 contraction chunks over the embedding dim
NQ = S // QT     # 4 q tiles
NST = S // 128   # 16 s tiles of 128
NSLAB = 8        # yTun ring depth (q-tile x head slabs)


class SplitDrainTileContext(tile.TileContext):
    """Kernel-tail drain with its sem waits split one per instruction.

    The walrus build here rejects instructions carrying more sync waits
    than their ISA struct encodes; TileContext hangs one wait per live
    proc on a single Drain. Sequential single-wait drains on the sync
    engine give the same guarantee.
    """

    def _drain_and_barrier(self, tick_clock, wait_clock):
        gc = list(tick_clock.global_clock)
        n = len(gc)
        for i, t in enumerate(gc):
            if t:
                vc = VectorClock([t if j == i else 0 for j in range(n)])
                inst = self.nc.sync.drain()
                wait_clock.add_sem_waits(inst.ins, ScopedClock({None: vc}))
        self.nc.all_engine_barrier()
        assert self.sems is not None
        popped = self.nc._tile_sem_poison_stack.pop()
        assert popped is self._sem_poison
        self.nc.clear_and_free_semaphores(list(self.sems.allocated().values()))
        self.nc.all_engine_barrier()


# ---------------------------------------------------------------- BIR fix

_CAPS = {"EventSemaphore": 2}
_DEFAULT_CAP = 1
_counter = [0]


def _split_bir_waits(bir):
    """Move excess sync waits onto EventSemaphores inserted just before
    the overloaded instruction (same engine => same program order)."""
    n = 0
    for fn in bir.get("functions", []):
        for bb in fn.get("blocks", []):
            out = []
            for inst in bb.get("instructions", []):
                si = inst.get("sync_info")
                waits = si.get("on_wait") if si else None
                cap = _CAPS.get(inst.get("opcode"), _DEFAULT_CAP)
                if waits and len(waits) > cap:
                    excess, keep = waits[:-cap], waits[-cap:]
                    for i in range(0, len(excess), 2):
                        _counter[0] += 1
                        out.append({
                            "debug": inst.get("debug", 0),
                            "engine": inst["engine"],
                            "ins": [], "outs": [],
                            "name": f"antsplitw-{_counter[0]}",
                            "opcode": "EventSemaphore",
                            "sync_info": {"on_update": [],
                                          "on_wait": excess[i:i + 2]},
                        })
                        n += 1
                    si["on_wait"] = keep
                out.append(inst)
            bb["instructions"] = out
    return n


def _install_bir_fix():
    import json
    import concourse.bass2jax as bass2jax
    from concourse.bass_utils import compile_bir_kernel as orig
    if getattr(bass2jax.compile_bir_kernel, "_ant_split", False):
        return

    def wrapped(ant_bir_str, *args, **kwargs):
        bir = json.loads(ant_bir_str)
        if _split_bir_waits(bir):
            ant_bir_str = json.dumps(bir).encode()
        return orig(ant_bir_str, *args, **kwargs)

    wrapped._ant_split = True
    bass2jax.compile_bir_kernel = wrapped


# ---------------------------------------------------------------- device

def build():
    nc = bass.Bass("TRN2", target_bir_lowering=False, debug=False)
    # all inputs host-repacked partition-major: [128, chunk, cols], so every
    # DMA line is a contiguous run >= 2 KB
    xT_d = nc.dram_tensor("xT", [128, EC, S], F16, kind="ExternalInput").ap()
    wqk_d = nc.dram_tensor("wqk", [128, EC, 512], F16, kind="ExternalInput").ap()
    wv_d = nc.dram_tensor("wv", [128, EC, 256], F16, kind="ExternalInput").ap()
    wp_d = nc.dram_tensor("wproj", [128, 2, E], F32R, kind="ExternalInput").ap()
    y_d = nc.dram_tensor("y", [S, E], F16, kind="ExternalOutput").ap()

    with SplitDrainTileContext(nc) as tc:
        with (
            tc.tile_pool(name="persist", bufs=1) as persist,
            tc.tile_pool(name="ptp", bufs=6) as ptp,
            tc.tile_pool(name="nrm", bufs=2) as nrm,
            tc.tile_pool(name="pout", bufs=4) as pout,
            tc.tile_pool(name="ps1", bufs=2, space="PSUM") as ps1,
            tc.tile_pool(name="pss", bufs=2, space="PSUM") as pss,
            tc.tile_pool(name="pav", bufs=2, space="PSUM") as pav,
        ):
            xT_sb = persist.tile([128, EC, S], F16)
            wqk_sb = persist.tile([128, EC, 512], F16)
            wv_sb = persist.tile([128, EC, 256], F16)
            wp_sb = persist.tile([128, 2, E], F32R)
            qT_sb = persist.tile([128, 2, S], F16)    # heads 01 | 23 stacked
            kTpad = persist.tile([128, HPC, S], F16)  # per head, half rows zero
            vaug = persist.tile([128, NST, HPC, D + 1], F16)
            yT = persist.tile([128, 2, S], F32R)      # normalized, proj lhsT
            yTun = persist.tile([65, NSLAB, QT], F32)  # unnormalized + rowsum
            ones = persist.tile([128, 64], F32R)

            # ---- input DMA kickoff, consumption-ordered on 3 queues ----
            def xt_piece(eng, ec, hf):
                eng.dma_start(
                    xT_sb[:, ec, hf * 1024:(hf + 1) * 1024],
                    xT_d[:, ec, hf * 1024:(hf + 1) * 1024])

            def wqk_piece(eng, e2):
                eng.dma_start(wqk_sb[:, 2 * e2:2 * e2 + 2, :],
                              wqk_d[:, 2 * e2:2 * e2 + 2, :])

            wqk_piece(nc.sync, 0)
            xt_piece(nc.sync, 0, 0)
            xt_piece(nc.sync, 2, 0)
            wqk_piece(nc.sync, 2)
            xt_piece(nc.sync, 4, 0)
            xt_piece(nc.sync, 6, 0)
            xt_piece(nc.sync, 1, 1)
            xt_piece(nc.sync, 5, 1)

            wqk_piece(nc.scalar, 1)
            xt_piece(nc.scalar, 1, 0)
            xt_piece(nc.scalar, 3, 0)
            wqk_piece(nc.scalar, 3)
            xt_piece(nc.scalar, 5, 0)
            xt_piece(nc.scalar, 7, 0)
            xt_piece(nc.scalar, 3, 1)
            xt_piece(nc.scalar, 7, 1)

            nc.gpsimd.dma_start(wv_sb[:], wv_d[:])
            for ec in (0, 2, 4, 6):
                xt_piece(nc.gpsimd, ec, 1)
            nc.gpsimd.dma_start(wp_sb[:], wp_d[:])

            nc.vector.memset(ones[:].bitcast(F32), 1.0)
            nc.vector.memset(vaug[:, :, :, D:D + 1].bitcast(U16), 15360)
            for h in range(HPC):
                dead = slice(64, 128) if h % 2 == 0 else slice(0, 64)
                nc.vector.memset(kTpad[dead, h, :].bitcast(U16), 0)

            # ---- stage emitters ----
            def qk_copy_out(rt, q4, ps_):
                sslc = slice(q4 * QT, (q4 + 1) * QT)
                if rt < 2:
                    nc.scalar.copy(qT_sb[:, rt, sslc], ps_)
                else:
                    h2 = 2 * (rt - 2)
                    nc.scalar.copy(kTpad[0:64, h2, sslc], ps_[0:64, :])
                    nc.vector.tensor_copy(out=kTpad[64:128, h2 + 1, sslc],
                                          in_=ps_[64:128, :])

            def qk_ec_major():
                # q/k projection for q tiles 0,1 with all 8 PSUM banks as
                # accumulators: each arriving xT chunk feeds all 8 matmuls
                ssA = pss.tile([128, 2, QT], F32, tag="ss", name="qkA")
                ssB = pss.tile([128, 2, QT], F32, tag="ss", name="qkB")
                m0 = ps1.tile([128, QT], F32, tag="m", name="qkm0")
                m1 = ps1.tile([128, QT], F32, tag="m", name="qkm1")
                a0 = pav.tile([128, QT], F32, tag="av", name="qka0")
                a1 = pav.tile([128, QT], F32, tag="av", name="qka1")
                accs = {(0, 0): ssA[:, 0, :], (0, 1): ssA[:, 1, :],
                        (1, 0): ssB[:, 0, :], (1, 1): ssB[:, 1, :],
                        (2, 0): m0[:], (2, 1): m1[:],
                        (3, 0): a0[:], (3, 1): a1[:]}
                for ec in range(EC):
                    for rt in range(4):
                        for q4 in range(2):
                            nc.tensor.matmul(
                                accs[(rt, q4)],
                                wqk_sb[:, ec, rt * 128:(rt + 1) * 128],
                                xT_sb[:, ec, q4 * QT:(q4 + 1) * QT],
                                start=(ec == 0), stop=(ec == EC - 1))
                for rt in range(4):
                    for q4 in range(2):
                        qk_copy_out(rt, q4, accs[(rt, q4)])

            def qk_rt_pair(rt, q4a, q4b):
                # one 128-col block of wqk against two moving q tiles,
                # ec-interleaved so each arriving xT chunk feeds 2 matmuls
                pa = ps1.tile([128, QT], F32, tag="m", name=f"pqk{rt}{q4a}")
                pb = ps1.tile([128, QT], F32, tag="m", name=f"pqk{rt}{q4b}")
                for ec in range(EC):
                    for ps_, q4 in ((pa, q4a), (pb, q4b)):
                        nc.tensor.matmul(
                            ps_[:], wqk_sb[:, ec, rt * 128:(rt + 1) * 128],
                            xT_sb[:, ec, q4 * QT:(q4 + 1) * QT],
                            start=(ec == 0), stop=(ec == EC - 1))
                for ps_, q4 in ((pa, q4a), (pb, q4b)):
                    qk_copy_out(rt, q4, ps_[:])

            def v_block(st2):
                pv = ps1.tile([128, 256], F32, tag="m", name=f"pv{st2}")
                for ec in range(EC):
                    nc.tensor.matmul(
                        pv[:], xT_sb[:, ec, st2 * 128:(st2 + 1) * 128],
                        wv_sb[:, ec, :],
                        start=(ec == 0), stop=(ec == EC - 1))
                nc.vector.tensor_copy(
                    out=vaug[:, st2, :, 0:D],
                    in_=pv[:, :].rearrange("p (h d) -> p h d", h=HPC))

            def attn_head(qj, h):
                qslc = slice(qj * QT, (qj + 1) * QT)
                qT_ap = qT_sb[:, h // 2, qslc]
                nkc = (qj + 1) * QT // KC
                av = pav.tile([65, QT], F32, tag="av", name=f"av{qj}{h}")
                for pr in range(nkc // 2):
                    ps = pss.tile([128, 2, QT], F32, tag="ss",
                                  name=f"ss{qj}{h}{pr}")
                    offs = [max(0, (2 * pr + j) * KC - qj * QT)
                            for j in range(2)]
                    for j in range(2):
                        kc = 2 * pr + j
                        o = offs[j]
                        nc.tensor.matmul(
                            ps[:, j, o:QT],
                            kTpad[:, h, kc * KC:(kc + 1) * KC],
                            qT_ap[:, o:QT],
                            start=True, stop=True)
                    pt = ptp.tile([128, 2, QT], F16, tag="pt",
                                  name=f"pt{qj}{h}{pr}")
                    if sum(offs) < 352:
                        # one exp for the pair; any dead-region garbage is
                        # never read (AV slices [o:])
                        nc.scalar.activation(
                            pt[:], ps[:],
                            mybir.ActivationFunctionType.Exp, scale=0.125)
                    else:
                        # deep-diagonal pair: exp only live columns
                        for j, o in enumerate(offs):
                            nc.scalar.activation(
                                pt[:, j, o:QT], ps[:, j, o:QT],
                                mybir.ActivationFunctionType.Exp, scale=0.125)
                    for j in range(2):
                        kc = 2 * pr + j
                        if kc * KC >= qj * QT:
                            # mask only the 128-wide diagonal band
                            o = offs[j]
                            w = min(KC, QT - o)
                            nc.gpsimd.affine_select(
                                out=pt[:, j, o:o + w],
                                in_=pt[:, j, o:o + w],
                                compare_op=mybir.AluOpType.is_ge,
                                fill=0.0, base=qj * QT + o - kc * KC,
                                pattern=[[1, w]],
                                channel_multiplier=-1)
                    for j in range(2):
                        kc = 2 * pr + j
                        o = offs[j]
                        nc.tensor.matmul(av[:, o:QT],
                                         vaug[:, kc, h, :],
                                         pt[:, j, o:QT],
                                         start=(kc == 0),
                                         stop=(kc == nkc - 1))
                slab = (qj * HPC + h) % NSLAB
                nc.vector.tensor_copy(out=yTun[:, slab, :], in_=av[:])

            def recip_half(qj, half):
                # 1/rowsum for one head pair; exp(-ln(x)) = 1/x keeps the
                # Ln/Exp pair inside the exp activation table set
                t0 = (qj * HPC + 2 * half) % NSLAB
                rs2 = nrm.tile([2, QT], F32, tag="rs2", name=f"rs{qj}{half}")
                nc.sync.dma_start(rs2[:, :], yTun[64:65, t0:t0 + 2, :])
                lg = nrm.tile([2, QT], F32, tag="lg", name=f"lg{qj}{half}")
                nc.scalar.activation(lg[:, :], rs2[:, :],
                                     mybir.ActivationFunctionType.Ln)
                rt2 = nrm.tile([2, QT], F32R, tag="rt2", name=f"rt{qj}{half}")
                nc.scalar.activation(rt2[:, :], lg[:, :],
                                     mybir.ActivationFunctionType.Exp,
                                     scale=-1.0)
                rt_ts = []
                for i in range(2):
                    rt_t = nrm.tile([1, QT], F32R, tag=f"rt_{i}",
                                    name=f"rtt{qj}{half}{i}")
                    nc.sync.dma_start(rt_t[:, :], rt2[i:i + 1, :])
                    rt_ts.append(rt_t[:, :])
                return rt_ts

            def recip_fast(qj, half):
                # latency-lean variant for the kernel tail: Ln/Exp straight
                # on the rowsum row (1 partition), no DMA hops
                t0 = (qj * HPC + 2 * half) % NSLAB
                lgf = nrm.tile([1, 2, QT], F32, tag="lgf",
                               name=f"lgf{qj}{half}")
                nc.scalar.activation(lgf[:], yTun[64:65, t0:t0 + 2, :],
                                     mybir.ActivationFunctionType.Ln)
                rtf = nrm.tile([1, 2, QT], F32R, tag="rtf",
                               name=f"rtf{qj}{half}")
                nc.scalar.activation(rtf[:], lgf[:],
                                     mybir.ActivationFunctionType.Exp,
                                     scale=-1.0)
                return [rtf[:, 0, :], rtf[:, 1, :]]

            def norm_pair(qj, half, rt_ts):
                qslc = slice(qj * QT, (qj + 1) * QT)
                for i in range(2):
                    h = 2 * half + i
                    slab = (qj * HPC + h) % NSLAB
                    bc = ps1.tile([64, QT], F32, tag="m", name=f"bc{qj}{h}")
                    nc.tensor.matmul(bc[:], ones[0:1, 0:64],
                                     rt_ts[i], start=True, stop=True)
                    po = 64 * (h % 2)
                    with nc.allow_low_precision(reason="proj lhsT"):
                        nc.vector.tensor_tensor(
                            out=yT[po:po + 64, h // 2, qslc],
                            in0=yTun[0:64, slab, :], in1=bc[:],
                            op=mybir.AluOpType.mult)

            _oq = [0]
            _out_rot = (nc.sync, nc.gpsimd)

            def proj_qt(qt, rot=None, ceng=None, on_ss=False):
                # both 512-wide halves of one output row block; at the tail
                # (on_ss) a single 2-bank psum tile: one copy, one 256 KB DMA
                if on_ss:
                    pp = pss.tile([128, 2, QT], F32, tag="ss", name=f"pp{qt}")
                    halves = [pp[:, 0, :], pp[:, 1, :]]
                else:
                    halves = [ps1.tile([128, QT], F32, tag="m",
                                       name=f"pp{qt}{eo}")[:]
                              for eo in range(2)]
                for eo in range(2):
                    for ci in range(2):
                        nc.tensor.matmul(
                            halves[eo], yT[:, ci, qt * 128:(qt + 1) * 128],
                            wp_sb[:, ci, eo * 512:(eo + 1) * 512],
                            start=(ci == 0), stop=(ci == 1))
                po_t = pout.tile([128, 2 * QT], F16, tag="po", name=f"po{qt}")
                for eo in range(2):
                    oslc = po_t[:, eo * QT:(eo + 1) * QT]
                    if ceng is nc.scalar:
                        ceng.copy(oslc, halves[eo])
                    else:
                        nc.vector.tensor_copy(out=oslc, in_=halves[eo])
                rot = rot if rot is not None else _out_rot
                eng = rot[_oq[0] % len(rot)]
                _oq[0] += 1
                eng.dma_start(y_d[qt * 128:(qt + 1) * 128, :], po_t[:])

            # ---- merged pipeline ----
            # q/k for tiles 0,1 streams ec-major against arriving input
            qk_ec_major()
            # v for key tiles 0..7 (first-half xT columns only)
            for st2 in range(8):
                v_block(st2)
            # qk for q tiles 2,3 interleaved with attention on q tile 1
            for rt in range(4):
                qk_rt_pair(rt, 2, 3)
                attn_head(1, rt)
                if rt % 2 == 1:
                    norm_pair(1, rt // 2, recip_half(1, rt // 2))
            # attention q tile 2 + v tiles 8..15 + projection q tile 1
            for st2 in range(8, 12):
                v_block(st2)
            attn_head(2, 0)
            v_block(12)
            v_block(13)
            attn_head(2, 1)
            norm_pair(2, 0, recip_half(2, 0))
            v_block(14)
            v_block(15)
            attn_head(2, 2)
            proj_qt(4)
            proj_qt(5)
            attn_head(2, 3)
            norm_pair(2, 1, recip_half(2, 1))
            proj_qt(6)
            proj_qt(7)
            # attention q tile 3 + projection q tile 2
            for h in range(4):
                attn_head(3, h)
                if h % 2 == 1:
                    norm_pair(3, h // 2, recip_half(3, h // 2))
                proj_qt(8 + h)
            # tail: attention q tile 0 (cheapest) + projections 3 and 0
            attn_head(0, 0)
            proj_qt(12)
            attn_head(0, 1)
            norm_pair(0, 0, recip_half(0, 0))
            proj_qt(13)
            attn_head(0, 2)
            proj_qt(14)
            attn_head(0, 3)
            proj_qt(15)
            norm_pair(0, 1, recip_fast(0, 1))
            for qt in range(4):
                proj_qt(qt, rot=(nc.sync, nc.gpsimd), ceng=nc.scalar,
                        on_ss=True)
    return nc


# ---------------------------------------------------------------- host

_NC_CACHE = []


def _get_nc():
    if not _NC_CACHE:
        _install_bir_fix()
        _NC_CACHE.append(build())
    return _NC_CACHE[0]


def make_in_maps(x, w_attn, w_proj):
    in_maps = []
    for c in range(N_CORES):
        b, h0 = c // 4, (c % 4) * HPC
        wq = w_attn[:, h0 * D:(h0 + HPC) * D]
        wk = w_attn[:, E + h0 * D:E + (h0 + HPC) * D]
        wv = w_attn[:, 2 * E + h0 * D:2 * E + (h0 + HPC) * D]
        def pmaj(a, dt):
            # [n*128, c] row-chunked -> partition-major [128, n, c]
            n = a.shape[0] // 128
            return np.ascontiguousarray(
                a.reshape(n, 128, -1).transpose(1, 0, 2).astype(dt))

        in_maps.append({
            "xT": pmaj(x[b].T, np.float16),
            "wqk": pmaj(np.concatenate([wq, wk], axis=1), np.float16),
            "wv": pmaj(wv, np.float16),
            "wproj": pmaj(w_proj[h0 * D:(h0 + HPC) * D, :], np.float32),
        })
    return in_maps


def run(x, w_attn, w_proj, trace=False, tmpdir=None):
    from concourse.bass_utils import run_bass_kernel_spmd
    nc = _get_nc()
    res = run_bass_kernel_spmd(nc, make_in_maps(x, w_attn, w_proj),
                               list(range(N_CORES)), trace=trace, tmpdir=tmpdir)
    y = np.zeros((B, S, E), np.float32)
    for c in range(N_CORES):
        y[c // 4] += res.results[c]["y"].astype(np.float32)
    return y, res


def kernel(x, w_attn, w_proj):
    y, _ = run(np.asarray(x, np.float32), np.asarray(w_attn, np.float32),
               np.asarray(w_proj, np.float32))
    return y

